# revision 13
# baseline (speedup 1.0000x reference)
"""Trainium2 Bass kernel for nn_ConstraintLoss (mse + dynamics/obstacle loss).

Data-parallel over 8 cores (131072 rows -> 16384/core). Per core the shard
is processed in 7 row-tiles (K rows/partition: 8,24,32,32,16,12,4);
partition p owns shard rows [128p, 128p+128) so every tile load is one
contiguous K*960B block per partition. Load DMA descriptors are emitted
ahead of the compute (interleaved so SWDGE semaphore reuse never blocks
the GpSimd queue) and the HBM stream runs gapless at ~360 GB/s read (the
per-core HBM cap). The small first tile fills the pipeline early; the
small last tiles keep the post-stream tail short.

Math per row (telescoped; see reference):
  x = p[0:160] as (40,4)[px,py,th,v]; u = p[160:240] as (40,2)[a,w]
  resid/DT = (x39 - x0)/DT - q,  q = [q_c, q_s, q_w, q_a]
    q_c = v0 cos th0 + sum_{j<39} v_j cos th_j   (q_s with sin)
    q_w = sum_j w_j, q_a = sum_j a_j             (j < 40)
  dyn_err = DT * ||resid/DT||
  obst_err = sum_{k,j} sqrt((px_j-ox_k)^2+(py_j-oy_k)^2) - 40*sum_k (r_k+2)^2
  out = mean(diff^2) + mean(dyn_err + obst_err)

Engine split (HW-measured; GpSimd TT steals the shared SBUF port from
2-source DVE ops, so GpSimd gets only the dy sub, sized to overlap DVE's
immune single-source phase):
  - DVE: range wraps (custom ADD_RANGE_WRAP), u-sums, dx, diff (2x,
    in-place into the t tile), custom SQSQADD (d2 = dx^2+dy^2), vcs (2x),
    reductions, batched phase B
  - GpSimd: dy broadcast sub + all DMA emissions
  - ACT: mse Square+accum, Sin, v-broadcast, X39/X0 snapshots, (r+2)^2,
    sqrt batches (tiles 0-3 mid-stream, 4-6 + dyn-norm at the end)
Trig: sin(th)=Sin(wrap(th)), cos(th)=Sin(wrap(th+pi/2)); wrap valid for
|arg|<3pi (|th|<6 here).
"""

from contextlib import ExitStack

import numpy as np

import concourse.bacc as bacc
import concourse.bass as bass
import concourse.tile as tile
import concourse.dve_ops as dve_ops
from concourse.dve_spec import Spec, Src0, Src1, sq
from concourse import mybir
from concourse.bass_utils import run_bass_kernel_spmd

N_CORES = 8
B = 131072
BC = B // N_CORES            # 16384 rows per core
P = 128                      # SBUF partitions
RPP = BC // P                # 128 rows per partition
KS = [8, 32, 32, 32, 16, 8]
OFFS = [0, 8, 40, 72, 104, 120]
NT = len(KS)
DT = 0.25
CAR_WIDTH = 2.0
N_OBST = 3
NJ = 40
PI = float(np.pi)
TWO_PI = float(2.0 * np.pi)
HALF_PI = float(np.pi / 2.0)
F32 = mybir.dt.float32
BF16 = mybir.dt.bfloat16
KMAX = 32
GKJ = N_OBST * NJ            # 120 dist values per row
NSPLIT = 5                   # sqrt batch 1 covers tiles [0, NSPLIT)
PHB_T = NT - 1               # phase-B part A covers tiles [0, PHB_T)

OUT_COLS = RPP + NT + 2  # 128 + 6 + 2 = 136


def _bcast(ap, dim_idx, count):
    """Insert a step-0 (broadcast) dim at position dim_idx of ap's dim list."""
    dims = [list(d) for d in ap.ap]
    dims.insert(dim_idx, [0, count])
    return bass.AP(tensor=ap.tensor, offset=ap.offset, ap=dims)


def _register_custom(name, body, reference, shas):
    """Register a custom DveOp (idempotent); discover shas if pinned ones drift."""
    for op in dve_ops.OPS:
        if op.name == name:
            return op
    spec = Spec(body=body, reference=reference)
    row = dve_ops._CUSTOM_DVE_ROW_BASE + len(dve_ops.OPS)
    dve_ops._SUB_OPCODE_FOR_NAME[name] = row
    op = dve_ops.DveOp(name, spec, False, dict(shas))
    import re
    for ver in ("v3", "v4"):
        try:
            op.compile(ver)
        except ValueError as e:
            m = re.search(r"v\d: (\w{16})", str(e))
            shas = dict(shas)
            shas[ver] = m.group(1)
            dve_ops._COMPILE_CACHE.pop((name, ver), None)
            op = dve_ops.DveOp(name, spec, False, shas)
    dve_ops.OPS.append(op)
    dve_ops.CUSTOM_DVE_SPECS[name] = spec
    return op


def _sqsqadd():
    return _register_custom(
        "SQSQADD_ANT", sq(Src0) + sq(Src1),
        lambda in0, in1, s0, s1, imm2: (
            in0.astype(np.float32) ** 2 + in1.astype(np.float32) ** 2),
        {"v3": "cd4bd6e1c27efd14", "v4": "121e32d8332f5047"})


def build_nc():
    sqsqadd = _sqsqadd()
    nc = bacc.Bacc()
    pred = nc.declare_dram_parameter("predictions", [BC, 240], F32, isOutput=False)
    tgt = nc.declare_dram_parameter("targets", [BC, 240], F32, isOutput=False)
    inp = nc.declare_dram_parameter("inputs", [BC, 13], F32, isOutput=False)
    out = nc.declare_dram_parameter("out", [P, OUT_COLS], F32, isOutput=True)

    predv = pred[:].rearrange("(p r) c -> p r c", p=P, r=RPP)
    tgtv = tgt[:].rearrange("(p r) c -> p r c", p=P, r=RPP)
    inpv = inp[:].rearrange("(p r) c -> p r c", p=P, r=RPP)

    SPLIT = OFFS[NSPLIT] * GKJ       # 11520
    PHB_O = OFFS[PHB_T]              # 124

    with tile.TileContext(nc) as tc, ExitStack() as ctx:
        per = ctx.enter_context(tc.tile_pool(name="per", bufs=1))
        ws_p = ctx.enter_context(tc.tile_pool(name="wsp", bufs=1))
        cs_p = ctx.enter_context(tc.tile_pool(name="csp", bufs=1))
        vcs_p = ctx.enter_context(tc.tile_pool(name="vcsp", bufs=2))
        dxy_p = ctx.enter_context(tc.tile_pool(name="dxyp", bufs=2))
        tp = ctx.enter_context(tc.tile_pool(name="tp", bufs=3))
        sm_p = ctx.enter_context(tc.tile_pool(name="smp", bufs=1))

        # per-tile p/i buffers (own allocations); t tiles pool-share 2 bufs
        pts, its = [], []
        for t in range(NT):
            K = KS[t]
            pts.append(per.tile([P, K, 240], BF16, name=f"p_{t}"))
            its.append(per.tile([P, K, 13], BF16, name=f"i_{t}"))

        tts = {}

        def emit_loads(t):
            o = OFFS[t]
            K = KS[t]
            if t >= NT - 1:
                tts[t] = per.tile([P, K, 240], BF16, name=f"t_{t}")
            else:
                tts[t] = tp.tile([P, KMAX, 240], BF16, name=f"t_{t}", tag="t")
            nc.gpsimd.dma_start(out=pts[t][:], in_=predv[:, o:o + K])
            nc.gpsimd.dma_start(out=its[t][:], in_=inpv[:, o:o + K])
            nc.gpsimd.dma_start(out=tts[t][:, 0:K], in_=tgtv[:, o:o + K])

        emit_loads(0)
        emit_loads(1)
        emit_loads(2)

        CPOS = per.tile([P, 1], F32)
        CW = per.tile([P, 1], F32)
        nc.vector.memset(CPOS[:], HALF_PI)
        nc.vector.memset(CW[:], CAR_WIDTH)
        TRASH1 = per.tile([P, 1], F32)
        nc.scalar.activation(out=TRASH1[:], in_=CPOS[:],
                             func=mybir.ActivationFunctionType.Sin)
        ws0 = ws_p.tile([P, KMAX, 2, 40], BF16, name="ws_init", tag="ws")
        cs0 = cs_p.tile([P, KMAX, 2, 40], BF16, name="cs_init", tag="cs")
        nc.vector.memset(ws0[:, :, :, 39:40], 0.0)
        nc.vector.memset(cs0[:, :, :, 39:40], 0.0)

        Q = per.tile([P, RPP, 4], F32)
        X39 = per.tile([P, RPP, 4], F32)
        X0 = per.tile([P, RPP, 4], F32)
        X0W = per.tile([P, RPP, 2], F32)
        X0CS = per.tile([P, RPP, 2], BF16)
        X0M = per.tile([P, RPP, 2], BF16)
        D2 = per.tile([P, RPP * GKJ], BF16)
        DY2 = per.tile([P, RPP], F32)
        RES = per.tile([P, RPP, 4], F32)
        OUT = per.tile([P, OUT_COLS], F32)

        def phase_b(lo, hi):
            """resid -> DY2 for row-groups [lo, hi)."""
            s = slice(lo, hi)
            nc.vector.tensor_mul(out=X0M[:, s], in0=X0CS[:, s],
                                 in1=_bcast(X0[:, s, 3], 2, 2))
            nc.vector.tensor_add(out=Q[:, s, 0:2], in0=Q[:, s, 0:2],
                                 in1=X0M[:, s])
            nc.vector.tensor_sub(out=RES[:, s], in0=X39[:, s], in1=X0[:, s])
            nc.vector.scalar_tensor_tensor(
                out=X39[:, s], in0=RES[:, s], scalar=1.0 / DT, in1=Q[:, s],
                op0=mybir.AluOpType.mult, op1=mybir.AluOpType.subtract)
            nc.scalar.activation(out=X39[:, s], in_=X39[:, s],
                                 func=mybir.ActivationFunctionType.Square)
            nc.vector.reduce_sum(out=DY2[:, s], in_=X39[:, s],
                                 axis=mybir.AxisListType.X)

        for t in range(NT):
            K, p_t, t_t, i_t = KS[t], pts[t], tts[t], its[t]
            o = OFFS[t]
            ts = slice(o, o + K)
            tv = t_t[:, 0:K]

            xv = p_t[:, :, 0:160].rearrange("p g (j f) -> p g j f", f=4)
            uv = p_t[:, :, 160:240].rearrange("p g (j f) -> p g j f", f=2)
            ov = i_t[:, :, 4:13].rearrange("p g (k f) -> p g k f", f=3)
            th38 = xv[:, :, 0:39, 2]
            v38 = xv[:, :, 0:39, 3]

            ws = ws_p.tile([P, KMAX, 2, 40], BF16, name=f"ws_{t}", tag="ws")
            cs = cs_p.tile([P, KMAX, 2, 40], BF16, name=f"cs_{t}", tag="cs")
            vcs = vcs_p.tile([P, KMAX, 2, 40], BF16, name=f"vcs_{t}", tag="vcs")
            dxy = dxy_p.tile([P, KMAX * GKJ * 2], BF16, name=f"dxy_{t}", tag="dxy")

            # dxy pairs: (g, k, j, c) with c=(x,y) innermost -> 2x-mode subs
            dxyv = dxy[:, 0:K * GKJ * 2].rearrange(
                "p (g k j c) -> p g k j c", k=N_OBST, j=NJ, c=2)
            pxy = xv[:, :, :, 0:2]          # (p, g, j, f=2) step-1 inner
            OXY = sm_p.tile([P, KMAX, N_OBST, 2], BF16, name=f"oxy_{t}",
                            tag="oxy")

            # --- DVE single-source phase ---
            # one wrap; cos comes from Sin(pi/2 - |wrap|) via ACT Abs + affine
            nc.vector.add_range_wrap(out=ws[:, 0:K, 1, 0:39], in_=th38,
                                     shift=0.0, bound=PI, period=TWO_PI)
            nc.vector.add_range_wrap(out=X0W[:, ts, 1], in_=i_t[:, :, 2],
                                     shift=0.0, bound=PI, period=TWO_PI)
            nc.vector.reduce_sum(
                out=Q[:, ts, 2:3], in_=uv[:, :, :, 1:2].rearrange(
                    "p g j f -> p g f j"), axis=mybir.AxisListType.X)
            nc.vector.reduce_sum(
                out=Q[:, ts, 3:4], in_=uv[:, :, :, 0:1].rearrange(
                    "p g j f -> p g f j"), axis=mybir.AxisListType.X)

            # --- ACT (trig set) ---
            nc.scalar.activation(out=OXY[:, 0:K], in_=ov[:, :, :, 0:2],
                                 func=mybir.ActivationFunctionType.Identity)
            nc.scalar.activation(out=ws[:, 0:K, 0, 0:39], in_=ws[:, 0:K, 1, 0:39],
                                 func=mybir.ActivationFunctionType.Abs)
            nc.scalar.activation(out=cs[:, 0:K, 0, 0:39], in_=ws[:, 0:K, 0, 0:39],
                                 func=mybir.ActivationFunctionType.Sin,
                                 scale=-1.0, bias=CPOS[:, 0:1])
            nc.scalar.activation(out=cs[:, 0:K, 1, 0:39], in_=ws[:, 0:K, 1, 0:39],
                                 func=mybir.ActivationFunctionType.Sin)
            nc.scalar.activation(out=ws[:, 0:K, :, 0:39], in_=_bcast(v38, 2, 2),
                                 func=mybir.ActivationFunctionType.Identity)
            nc.scalar.activation(out=X0W[:, ts, 0], in_=X0W[:, ts, 1],
                                 func=mybir.ActivationFunctionType.Abs)
            nc.scalar.activation(out=X0CS[:, ts, 0], in_=X0W[:, ts, 0],
                                 func=mybir.ActivationFunctionType.Sin,
                                 scale=-1.0, bias=CPOS[:, 0:1])
            nc.scalar.activation(out=X0CS[:, ts, 1], in_=X0W[:, ts, 1],
                                 func=mybir.ActivationFunctionType.Sin)
            nc.scalar.activation(out=X39[:, ts, :], in_=xv[:, :, 39, :],
                                 func=mybir.ActivationFunctionType.Identity)
            nc.scalar.activation(out=X0[:, ts, :], in_=i_t[:, :, 0:4],
                                 func=mybir.ActivationFunctionType.Identity)

            # --- DVE two-source phase ---
            for k in range(N_OBST):
                nc.vector.tensor_sub(
                    out=dxyv[:, :, k], in0=pxy,
                    in1=_bcast(OXY[:, 0:K, k, :], 1, NJ))
            nc.vector.tensor_sub(out=tv, in0=p_t[:], in1=tv)
            dflat = dxy[:, 0:K * GKJ * 2]
            nc.vector._custom_dve(
                sqsqadd, out=D2[:, o * GKJ:(o + K) * GKJ],
                in0=dflat.rearrange("p (e c) -> p e c", c=2)[:, :, 0],
                in1=dflat.rearrange("p (e c) -> p e c", c=2)[:, :, 1])
            nc.vector.tensor_mul(out=vcs[:, 0:K], in0=cs[:, 0:K],
                                 in1=ws[:, 0:K])
            nc.vector.tensor_add(out=cs[:, 0:K, :, 0:20],
                                 in0=vcs[:, 0:K, :, 0:20],
                                 in1=vcs[:, 0:K, :, 20:40])
            nc.vector.tensor_add(out=cs[:, 0:K, :, 20:30],
                                 in0=cs[:, 0:K, :, 0:10],
                                 in1=cs[:, 0:K, :, 10:20])
            nc.vector.reduce_sum(out=Q[:, ts, 0:2], in_=cs[:, 0:K, :, 20:30],
                                 axis=mybir.AxisListType.X)

            # --- ACT: mse Square + accumulate over diff ---
            nc.scalar.activation(
                out=tv, in_=tv,
                func=mybir.ActivationFunctionType.Square,
                accum_out=OUT[:, RPP + t: RPP + t + 1])

            # prefetch emissions (placed after this tile's gpsimd op so the
            # SWDGE semaphore-reuse waits can't head-of-line block compute)
            if t + 3 < NT:
                emit_loads(t + 3)

            if t == PHB_T - 1:
                phase_b(0, PHB_O)

        phase_b(PHB_O, RPP)

        # sqrt phase: one table switch, batch 1 (tiles 0..NSPLIT-1) first
        nc.scalar.activation(
            out=D2[:, 0:SPLIT], in_=D2[:, 0:SPLIT],
            func=mybir.ActivationFunctionType.Sqrt,
            accum_out=OUT[:, RPP + NT: RPP + NT + 1])
        nc.scalar.activation(
            out=D2[:, SPLIT:], in_=D2[:, SPLIT:],
            func=mybir.ActivationFunctionType.Sqrt,
            accum_out=OUT[:, RPP + NT + 1: RPP + NT + 2])
        nc.scalar.activation(out=OUT[:, 0:RPP], in_=DY2[:],
                             func=mybir.ActivationFunctionType.Sqrt)

        nc.sync.dma_start(out=out[:], in_=OUT[:])

    nc.finalize()
    return nc


_NC_CACHE = None


def _get_nc():
    global _NC_CACHE
    if _NC_CACHE is None:
        _NC_CACHE = build_nc()
    return _NC_CACHE


def combine(outs, rad):
    """Host-side reduction of per-core partials (float64)."""
    dy = 0.0
    sq = 0.0
    ob = 0.0
    for o in outs:
        o = o.astype(np.float64)
        dy += o[:, 0:RPP].sum()
        sq += o[:, RPP:RPP + NT].sum()
        ob += o[:, RPP + NT:RPP + NT + 2].sum()
    mse = sq / (B * 240.0)
    constraint = (DT * dy + ob - NJ * rad) / B
    return np.float32(mse + constraint)


def kernel(predictions, targets, inputs):
    nc = _get_nc()
    preds = np.ascontiguousarray(predictions, dtype=np.float32).reshape(
        N_CORES, BC, 240)
    tgts = np.ascontiguousarray(targets, dtype=np.float32).reshape(
        N_CORES, BC, 240)
    inps = np.ascontiguousarray(inputs, dtype=np.float32).reshape(
        N_CORES, BC, 13)
    in_maps = [
        {"predictions": preds[c], "targets": tgts[c], "inputs": inps[c]}
        for c in range(N_CORES)
    ]
    rad = float(np.sum((inputs[:, 6:13:3].astype(np.float64) + CAR_WIDTH) ** 2))
    res = run_bass_kernel_spmd(nc, in_maps, core_ids=list(range(N_CORES)))
    return combine([r["out"] for r in res.results], rad)


# revision 14
# speedup vs baseline: 1.1661x; 1.1661x over previous
"""Trainium2 Bass kernel for nn_ConstraintLoss (mse + dynamics/obstacle loss).

Data-parallel over 8 cores (131072 rows -> 16384/core). Per core the shard
is processed in 7 row-tiles (K rows/partition: 8,24,32,32,16,12,4);
partition p owns shard rows [128p, 128p+128) so every tile load is one
contiguous K*960B block per partition. Load DMA descriptors are emitted
ahead of the compute (interleaved so SWDGE semaphore reuse never blocks
the GpSimd queue) and the HBM stream runs gapless at ~360 GB/s read (the
per-core HBM cap). The small first tile fills the pipeline early; the
small last tiles keep the post-stream tail short.

Math per row (telescoped; see reference):
  x = p[0:160] as (40,4)[px,py,th,v]; u = p[160:240] as (40,2)[a,w]
  resid/DT = (x39 - x0)/DT - q,  q = [q_c, q_s, q_w, q_a]
    q_c = v0 cos th0 + sum_{j<39} v_j cos th_j   (q_s with sin)
    q_w = sum_j w_j, q_a = sum_j a_j             (j < 40)
  dyn_err = DT * ||resid/DT||
  obst_err = sum_{k,j} sqrt((px_j-ox_k)^2+(py_j-oy_k)^2) - 40*sum_k (r_k+2)^2
  out = mean(diff^2) + mean(dyn_err + obst_err)

Engine split (HW-measured; GpSimd TT steals the shared SBUF port from
2-source DVE ops, so GpSimd gets only the dy sub, sized to overlap DVE's
immune single-source phase):
  - DVE: range wraps (custom ADD_RANGE_WRAP), u-sums, dx, diff (2x,
    in-place into the t tile), custom SQSQADD (d2 = dx^2+dy^2), vcs (2x),
    reductions, batched phase B
  - GpSimd: dy broadcast sub + all DMA emissions
  - ACT: mse Square+accum, Sin, v-broadcast, X39/X0 snapshots, (r+2)^2,
    sqrt batches (tiles 0-3 mid-stream, 4-6 + dyn-norm at the end)
Trig: sin(th)=Sin(wrap(th)), cos(th)=Sin(wrap(th+pi/2)); wrap valid for
|arg|<3pi (|th|<6 here).
"""

from contextlib import ExitStack

import numpy as np

import concourse.bacc as bacc
import concourse.bass as bass
import concourse.tile as tile
import concourse.dve_ops as dve_ops
from concourse.dve_spec import Spec, Src0, Src1, sq
from concourse import mybir
from concourse.bass_utils import run_bass_kernel_spmd

N_CORES = 8
B = 131072
BC = B // N_CORES            # 16384 rows per core
P = 128                      # SBUF partitions
RPP = BC // P                # 128 rows per partition
KS = [8, 32, 32, 32, 16, 8]
OFFS = [0, 8, 40, 72, 104, 120]
NT = len(KS)
DT = 0.25
CAR_WIDTH = 2.0
N_OBST = 3
NJ = 40
PI = float(np.pi)
TWO_PI = float(2.0 * np.pi)
HALF_PI = float(np.pi / 2.0)
F32 = mybir.dt.float32
BF16 = mybir.dt.bfloat16
KMAX = 32
GKJ = N_OBST * NJ            # 120 dist values per row
NSPLIT = 4                   # sqrt batches: [0,2) after t2, [2,4) after t4
PHB_T = NT - 1               # phase-B part A covers tiles [0, PHB_T)

OUT_COLS = RPP + NT + 3  # 128 + 6 + 3 = 137


def _bcast(ap, dim_idx, count):
    """Insert a step-0 (broadcast) dim at position dim_idx of ap's dim list."""
    dims = [list(d) for d in ap.ap]
    dims.insert(dim_idx, [0, count])
    return bass.AP(tensor=ap.tensor, offset=ap.offset, ap=dims)


def _register_custom(name, body, reference, shas):
    """Register a custom DveOp (idempotent); discover shas if pinned ones drift."""
    for op in dve_ops.OPS:
        if op.name == name:
            return op
    spec = Spec(body=body, reference=reference)
    row = dve_ops._CUSTOM_DVE_ROW_BASE + len(dve_ops.OPS)
    dve_ops._SUB_OPCODE_FOR_NAME[name] = row
    op = dve_ops.DveOp(name, spec, False, dict(shas))
    import re
    for ver in ("v3", "v4"):
        try:
            op.compile(ver)
        except ValueError as e:
            m = re.search(r"v\d: (\w{16})", str(e))
            shas = dict(shas)
            shas[ver] = m.group(1)
            dve_ops._COMPILE_CACHE.pop((name, ver), None)
            op = dve_ops.DveOp(name, spec, False, shas)
    dve_ops.OPS.append(op)
    dve_ops.CUSTOM_DVE_SPECS[name] = spec
    return op


def _sqsqadd():
    return _register_custom(
        "SQSQADD_ANT", sq(Src0) + sq(Src1),
        lambda in0, in1, s0, s1, imm2: (
            in0.astype(np.float32) ** 2 + in1.astype(np.float32) ** 2),
        {"v3": "cd4bd6e1c27efd14", "v4": "121e32d8332f5047"})


def build_nc():
    sqsqadd = _sqsqadd()
    nc = bacc.Bacc()
    pred = nc.declare_dram_parameter("predictions", [BC, 240], F32, isOutput=False)
    tgt = nc.declare_dram_parameter("targets", [BC, 240], F32, isOutput=False)
    inp = nc.declare_dram_parameter("inputs", [BC, 13], F32, isOutput=False)
    out = nc.declare_dram_parameter("out", [P, OUT_COLS], F32, isOutput=True)

    predv = pred[:].rearrange("(p r) c -> p r c", p=P, r=RPP)
    tgtv = tgt[:].rearrange("(p r) c -> p r c", p=P, r=RPP)
    inpv = inp[:].rearrange("(p r) c -> p r c", p=P, r=RPP)

    SPLIT = OFFS[NSPLIT] * GKJ       # 11520
    PHB_O = OFFS[PHB_T]              # 124

    with tile.TileContext(nc) as tc, ExitStack() as ctx:
        per = ctx.enter_context(tc.tile_pool(name="per", bufs=1))
        ws_p = ctx.enter_context(tc.tile_pool(name="wsp", bufs=1))
        cs_p = ctx.enter_context(tc.tile_pool(name="csp", bufs=1))
        vcs_p = ctx.enter_context(tc.tile_pool(name="vcsp", bufs=2))
        dxy_p = ctx.enter_context(tc.tile_pool(name="dxyp", bufs=2))
        tp = ctx.enter_context(tc.tile_pool(name="tp", bufs=3))
        sm_p = ctx.enter_context(tc.tile_pool(name="smp", bufs=1))

        # per-tile p/i buffers (own allocations); t tiles pool-share 2 bufs
        pts, its = [], []
        for t in range(NT):
            K = KS[t]
            pts.append(per.tile([P, K, 240], BF16, name=f"p_{t}"))
            its.append(per.tile([P, K, 13], BF16, name=f"i_{t}"))

        tts = {}

        def emit_loads(t):
            o = OFFS[t]
            K = KS[t]
            if t >= NT - 1:
                tts[t] = per.tile([P, K, 240], BF16, name=f"t_{t}")
            else:
                tts[t] = tp.tile([P, KMAX, 240], BF16, name=f"t_{t}", tag="t")
            nc.gpsimd.dma_start(out=pts[t][:], in_=predv[:, o:o + K])
            nc.gpsimd.dma_start(out=its[t][:], in_=inpv[:, o:o + K])
            nc.gpsimd.dma_start(out=tts[t][:, 0:K], in_=tgtv[:, o:o + K])

        emit_loads(0)
        emit_loads(1)
        emit_loads(2)

        CPOS = per.tile([P, 1], F32)
        CW = per.tile([P, 1], F32)
        nc.vector.memset(CPOS[:], HALF_PI)
        nc.vector.memset(CW[:], CAR_WIDTH)
        TRASH1 = per.tile([P, 1], F32)
        nc.scalar.activation(out=TRASH1[:], in_=CPOS[:],
                             func=mybir.ActivationFunctionType.Sin)
        ws0 = ws_p.tile([P, KMAX, 2, 40], BF16, name="ws_init", tag="ws")
        cs0 = cs_p.tile([P, KMAX, 2, 40], BF16, name="cs_init", tag="cs")
        nc.vector.memset(ws0[:, :, :, 39:40], 0.0)
        nc.vector.memset(cs0[:, :, :, 39:40], 0.0)

        Q = per.tile([P, RPP, 4], F32)
        X39 = per.tile([P, RPP, 4], F32)
        X0 = per.tile([P, RPP, 4], F32)
        X0W = per.tile([P, RPP, 2], F32)
        X0CS = per.tile([P, RPP, 2], BF16)
        X0M = per.tile([P, RPP, 2], BF16)
        D2 = per.tile([P, RPP * GKJ], BF16)
        DY2 = per.tile([P, RPP], F32)
        RES = per.tile([P, RPP, 4], F32)
        OUT = per.tile([P, OUT_COLS], F32)

        def phase_b(lo, hi):
            """resid -> DY2 for row-groups [lo, hi)."""
            s = slice(lo, hi)
            nc.vector.tensor_mul(out=X0M[:, s], in0=X0CS[:, s],
                                 in1=_bcast(X0[:, s, 3], 2, 2))
            nc.vector.tensor_add(out=Q[:, s, 0:2], in0=Q[:, s, 0:2],
                                 in1=X0M[:, s])
            nc.vector.tensor_sub(out=RES[:, s], in0=X39[:, s], in1=X0[:, s])
            nc.vector.scalar_tensor_tensor(
                out=X39[:, s], in0=RES[:, s], scalar=1.0 / DT, in1=Q[:, s],
                op0=mybir.AluOpType.mult, op1=mybir.AluOpType.subtract)
            nc.scalar.activation(out=X39[:, s], in_=X39[:, s],
                                 func=mybir.ActivationFunctionType.Square)
            nc.vector.reduce_sum(out=DY2[:, s], in_=X39[:, s],
                                 axis=mybir.AxisListType.X)

        for t in range(NT):
            K, p_t, t_t, i_t = KS[t], pts[t], tts[t], its[t]
            o = OFFS[t]
            ts = slice(o, o + K)
            tv = t_t[:, 0:K]

            xv = p_t[:, :, 0:160].rearrange("p g (j f) -> p g j f", f=4)
            uv = p_t[:, :, 160:240].rearrange("p g (j f) -> p g j f", f=2)
            ov = i_t[:, :, 4:13].rearrange("p g (k f) -> p g k f", f=3)
            th38 = xv[:, :, 0:39, 2]
            v38 = xv[:, :, 0:39, 3]

            ws = ws_p.tile([P, KMAX, 2, 40], BF16, name=f"ws_{t}", tag="ws")
            cs = cs_p.tile([P, KMAX, 2, 40], BF16, name=f"cs_{t}", tag="cs")
            vcs = vcs_p.tile([P, KMAX, 2, 40], BF16, name=f"vcs_{t}", tag="vcs")
            dxy = dxy_p.tile([P, KMAX * GKJ * 2], BF16, name=f"dxy_{t}", tag="dxy")

            # dxy pairs: (g, k, j, c) with c=(x,y) innermost -> 2x-mode subs
            dxyv = dxy[:, 0:K * GKJ * 2].rearrange(
                "p (g k j c) -> p g k j c", k=N_OBST, j=NJ, c=2)
            pxy = xv[:, :, :, 0:2]          # (p, g, j, f=2) step-1 inner
            OXY = sm_p.tile([P, KMAX, N_OBST, 2], BF16, name=f"oxy_{t}",
                            tag="oxy")

            # --- DVE single-source phase ---
            # one wrap; cos comes from Sin(pi/2 - |wrap|) via ACT Abs + affine
            nc.vector.add_range_wrap(out=ws[:, 0:K, 1, 0:39], in_=th38,
                                     shift=0.0, bound=PI, period=TWO_PI)
            nc.vector.add_range_wrap(out=X0W[:, ts, 1], in_=i_t[:, :, 2],
                                     shift=0.0, bound=PI, period=TWO_PI)
            nc.vector.reduce_sum(
                out=Q[:, ts, 2:3], in_=uv[:, :, :, 1:2].rearrange(
                    "p g j f -> p g f j"), axis=mybir.AxisListType.X)
            nc.vector.reduce_sum(
                out=Q[:, ts, 3:4], in_=uv[:, :, :, 0:1].rearrange(
                    "p g j f -> p g f j"), axis=mybir.AxisListType.X)

            # --- ACT (trig set) ---
            nc.scalar.activation(out=OXY[:, 0:K], in_=ov[:, :, :, 0:2],
                                 func=mybir.ActivationFunctionType.Identity)
            nc.scalar.activation(out=ws[:, 0:K, 0, 0:39], in_=ws[:, 0:K, 1, 0:39],
                                 func=mybir.ActivationFunctionType.Abs)
            nc.scalar.activation(out=cs[:, 0:K, 0, 0:39], in_=ws[:, 0:K, 0, 0:39],
                                 func=mybir.ActivationFunctionType.Sin,
                                 scale=-1.0, bias=CPOS[:, 0:1])
            nc.scalar.activation(out=cs[:, 0:K, 1, 0:39], in_=ws[:, 0:K, 1, 0:39],
                                 func=mybir.ActivationFunctionType.Sin)
            nc.scalar.activation(out=ws[:, 0:K, :, 0:39], in_=_bcast(v38, 2, 2),
                                 func=mybir.ActivationFunctionType.Identity)
            nc.scalar.activation(out=X0W[:, ts, 0], in_=X0W[:, ts, 1],
                                 func=mybir.ActivationFunctionType.Abs)
            nc.scalar.activation(out=X0CS[:, ts, 0], in_=X0W[:, ts, 0],
                                 func=mybir.ActivationFunctionType.Sin,
                                 scale=-1.0, bias=CPOS[:, 0:1])
            nc.scalar.activation(out=X0CS[:, ts, 1], in_=X0W[:, ts, 1],
                                 func=mybir.ActivationFunctionType.Sin)
            nc.scalar.activation(out=X39[:, ts, :], in_=xv[:, :, 39, :],
                                 func=mybir.ActivationFunctionType.Identity)
            nc.scalar.activation(out=X0[:, ts, :], in_=i_t[:, :, 0:4],
                                 func=mybir.ActivationFunctionType.Identity)

            # --- DVE two-source phase ---
            for k in range(N_OBST):
                nc.vector.tensor_sub(
                    out=dxyv[:, :, k], in0=pxy,
                    in1=_bcast(OXY[:, 0:K, k, :], 1, NJ))
            nc.vector.tensor_sub(out=tv, in0=p_t[:], in1=tv)
            dflat = dxy[:, 0:K * GKJ * 2]
            nc.vector._custom_dve(
                sqsqadd, out=D2[:, o * GKJ:(o + K) * GKJ],
                in0=dflat.rearrange("p (e c) -> p e c", c=2)[:, :, 0],
                in1=dflat.rearrange("p (e c) -> p e c", c=2)[:, :, 1])
            nc.vector.tensor_mul(out=vcs[:, 0:K], in0=cs[:, 0:K],
                                 in1=ws[:, 0:K])
            nc.vector.tensor_add(out=cs[:, 0:K, :, 0:20],
                                 in0=vcs[:, 0:K, :, 0:20],
                                 in1=vcs[:, 0:K, :, 20:40])
            nc.vector.tensor_add(out=cs[:, 0:K, :, 20:30],
                                 in0=cs[:, 0:K, :, 0:10],
                                 in1=cs[:, 0:K, :, 10:20])
            nc.vector.reduce_sum(out=Q[:, ts, 0:2], in_=cs[:, 0:K, :, 20:30],
                                 axis=mybir.AxisListType.X)

            # --- ACT: mse Square + accumulate over diff ---
            nc.scalar.activation(
                out=tv, in_=tv,
                func=mybir.ActivationFunctionType.Square,
                accum_out=OUT[:, RPP + t: RPP + t + 1])

            # prefetch emissions (placed after this tile's gpsimd op so the
            # SWDGE semaphore-reuse waits can't head-of-line block compute)
            if t + 3 < NT:
                emit_loads(t + 3)

            if t == 2:
                SP_A = OFFS[2] * GKJ
                nc.scalar.activation(
                    out=D2[:, 0:SP_A], in_=D2[:, 0:SP_A],
                    func=mybir.ActivationFunctionType.Sqrt,
                    accum_out=OUT[:, RPP + NT: RPP + NT + 1])
            if t == PHB_T - 1:
                SP_A = OFFS[2] * GKJ
                nc.scalar.activation(
                    out=D2[:, SP_A:SPLIT], in_=D2[:, SP_A:SPLIT],
                    func=mybir.ActivationFunctionType.Sqrt,
                    accum_out=OUT[:, RPP + NT + 1: RPP + NT + 2])
                phase_b(0, PHB_O)

        phase_b(PHB_O, RPP)

        # final sqrt batch (tiles NSPLIT..) + dyn-norm sqrt
        nc.scalar.activation(
            out=D2[:, SPLIT:], in_=D2[:, SPLIT:],
            func=mybir.ActivationFunctionType.Sqrt,
            accum_out=OUT[:, RPP + NT + 2: RPP + NT + 3])
        nc.scalar.activation(out=OUT[:, 0:RPP], in_=DY2[:],
                             func=mybir.ActivationFunctionType.Sqrt)

        nc.sync.dma_start(out=out[:], in_=OUT[:])

    nc.finalize()
    return nc


_NC_CACHE = None


def _get_nc():
    global _NC_CACHE
    if _NC_CACHE is None:
        _NC_CACHE = build_nc()
    return _NC_CACHE


def combine(outs, rad):
    """Host-side reduction of per-core partials (float64)."""
    dy = 0.0
    sq = 0.0
    ob = 0.0
    for o in outs:
        o = o.astype(np.float64)
        dy += o[:, 0:RPP].sum()
        sq += o[:, RPP:RPP + NT].sum()
        ob += o[:, RPP + NT:RPP + NT + 3].sum()
    mse = sq / (B * 240.0)
    constraint = (DT * dy + ob - NJ * rad) / B
    return np.float32(mse + constraint)


def kernel(predictions, targets, inputs):
    nc = _get_nc()
    preds = np.ascontiguousarray(predictions, dtype=np.float32).reshape(
        N_CORES, BC, 240)
    tgts = np.ascontiguousarray(targets, dtype=np.float32).reshape(
        N_CORES, BC, 240)
    inps = np.ascontiguousarray(inputs, dtype=np.float32).reshape(
        N_CORES, BC, 13)
    in_maps = [
        {"predictions": preds[c], "targets": tgts[c], "inputs": inps[c]}
        for c in range(N_CORES)
    ]
    rad = float(np.sum((inputs[:, 6:13:3].astype(np.float64) + CAR_WIDTH) ** 2))
    res = run_bass_kernel_spmd(nc, in_maps, core_ids=list(range(N_CORES)))
    return combine([r["out"] for r in res.results], rad)
